# revision 27
# baseline (speedup 1.0000x reference)
"""GCN decoder (2x GCNConv + MLP readout) on 8 Trainium2 NeuronCores.

Sharding: nodes are partitioned across the 8 cores by target-node range
(graph parallel). Edges are routed (on host) to the core that owns their
target node and sorted by target. Each core aggregates messages for its
own nodes with dma_gather + selection-matrix matmuls (scatter-add becomes
dense PE work); the layer-2 feature table is exchanged with one AllGather.

dma_gather uses int16 indices over 256B elements, so the feature tables
are stored as fp16 rows padded to 128 columns (256B) and edges are grouped
into blocks whose sources all come from one quarter of the table
(quarter size 25088 < int16 range).

self-contained: hardcodes the problem geometry, imports only installed
packages (numpy / concourse).
"""

import os
from contextlib import ExitStack

import numpy as np

# ---------------- fixed geometry ----------------
G = 8          # cores
NQ = 4         # table quarters (int16 index range for dma_gather)
WIN = 128      # nodes per PSUM accumulation window
SW = 32        # nodes per matmul output slice (sub-window)
KBLK = 128     # edge slots per matmul block
D = 64         # input feature dim
DP = 128       # padded row width (256B rows for dma_gather)
H = 64         # hidden dim
OUT = 32
HOR = 12
OUTF = OUT * HOR

# batching knobs
NB_DEG = 8     # windows per deg-reduce batch
NB_XP = 8      # windows per x-prescale batch
WG = 4         # windows per gather batch


def _cdiv(a, b):
    return (a + b - 1) // b


def _group_positions(sorted_group_ids, n_groups):
    counts = np.bincount(sorted_group_ids, minlength=n_groups)
    offsets = np.concatenate([[0], np.cumsum(counts)[:-1]])
    return np.arange(len(sorted_group_ids)) - offsets[sorted_group_ids], counts, offsets


def host_prep(x, edge_index, edge_weight):
    """Route/sort/pad edges, build CSR + slot arrays + static block schedule."""
    x = np.asarray(x, np.float32)
    edge_index = np.asarray(edge_index).astype(np.int64)
    edge_weight = np.asarray(edge_weight, np.float32)

    N = x.shape[0]
    NLOC = _cdiv(N, G)
    NPAD = _cdiv(NLOC, WIN) * WIN
    NTOT = G * NPAD
    QROWS = NTOT // NQ
    NWINL = NPAD // WIN
    NWINF = NTOT // WIN
    NSW = NPAD // SW
    assert NLOC < NPAD, "need dummy rows for gather padding"
    assert QROWS < 32768, "quarter must fit int16 index range"

    loop = np.arange(N, dtype=np.int64)
    row = np.concatenate([edge_index[0], loop])
    col = np.concatenate([edge_index[1], loop])
    ew = np.concatenate([edge_weight, np.ones(N, np.float32)])

    core_of = np.minimum(col // NLOC, G - 1)
    rcore = np.minimum(row // NLOC, G - 1)
    gid_row = (rcore * NPAD + (row - rcore * NLOC)).astype(np.int64)

    # ---- full-graph CSR of edge weights, gid order, for on-device degrees
    gcol = (core_of * NPAD + (col - core_of * NLOC)).astype(np.int64)
    order = np.argsort(gcol, kind="stable")
    gcol_s = gcol[order]
    pos, counts_all, _ = _group_positions(gcol_s, NTOT)
    K1 = max(8, int(counts_all.max()))
    K1 = _cdiv(K1, 8) * 8
    csr = np.zeros((NTOT, K1), np.float16)
    csr[gcol_s, pos] = ew[order].astype(np.float16)
    dummy_mask = np.zeros(NTOT, bool)
    for k in range(G):
        n_real = min(NLOC, max(0, N - k * NLOC))
        dummy_mask[k * NPAD + n_real : (k + 1) * NPAD] = True
    csr[dummy_mask, 0] = 1.0
    csr_pt = np.ascontiguousarray(csr.reshape(NWINF, WIN, K1).transpose(0, 2, 1))

    # ---- per-core edges sorted by (target sub-window, source quarter)
    per_core = []
    counts_swq = np.zeros((G, NSW * NQ), np.int64)
    for k in range(G):
        m = core_of == k
        e_gr = gid_row[m]
        e_cl = (col[m] - k * NLOC).astype(np.int64)
        e_w = ew[m].astype(np.float16)
        e_q = e_gr // QROWS
        key = (e_cl // SW) * NQ + e_q
        o = np.argsort(key, kind="stable")
        e_gr, e_cl, e_w, key = e_gr[o], e_cl[o], e_w[o], key[o]
        counts_swq[k] = np.bincount(key, minlength=NSW * NQ)
        per_core.append((e_gr, e_cl, e_w, key))

    # blocks per (sw, q): shared static schedule; q of each block is static
    Bq = _cdiv(counts_swq.max(axis=0), KBLK).reshape(NSW, NQ)  # may be 0
    # every sub-window needs >=1 block so its psum slice gets initialized;
    # self-loops guarantee the own-core quarter is nonempty, but be safe:
    for swi in range(NSW):
        if Bq[swi].sum() == 0:
            Bq[swi, 0] = 1

    # global block order: per gather batch (WG windows), quarter-major
    batches = []
    blk_sw = []
    blk_q = []
    blk_of_swq = {}
    for g0 in range(0, NWINL, WG):
        g1 = min(g0 + WG, NWINL)
        b_start = len(blk_sw)
        q_ranges = []
        for q in range(NQ):
            q_lo = len(blk_sw)
            for w in range(g0, g1):
                for swi in range(w * 4, w * 4 + 4):
                    nbq = int(Bq[swi, q])
                    if nbq:
                        blk_of_swq[(swi, q)] = (len(blk_sw), nbq)
                        for _ in range(nbq):
                            blk_sw.append(swi)
                            blk_q.append(q)
            q_ranges.append((q_lo, len(blk_sw)))
        batches.append(dict(g0=g0, g1=g1, blo=b_start, bhi=len(blk_sw),
                            q_ranges=q_ranges))
    TOTBLK = len(blk_sw)
    SLOTS = TOTBLK * KBLK
    blk_sw = np.asarray(blk_sw)
    blk_q = np.asarray(blk_q)
    # first/last block of each sub-window in global order
    block_first = np.zeros(TOTBLK, bool)
    block_last = np.zeros(TOTBLK, bool)
    seen = set()
    last_of = {}
    for b in range(TOTBLK):
        swi = int(blk_sw[b])
        if swi not in seen:
            seen.add(swi)
            block_first[b] = True
        last_of[swi] = b
    for b in last_of.values():
        block_last[b] = True
    win_blocks = [[] for _ in range(NWINL)]
    for b in range(TOTBLK):
        win_blocks[int(blk_sw[b]) // 4].append(b)

    core_arrays = []
    for k in range(G):
        e_gr, e_cl, e_w, key = per_core[k]
        # slot position for each edge: its (sw, q) block region + within-pos
        wpos, _, _ = _group_positions(key, NSW * NQ)
        base = np.zeros(NSW * NQ, np.int64)
        for (swi, q), (b0, nbq) in blk_of_swq.items():
            base[swi * NQ + q] = b0 * KBLK
        slot = base[key] + wpos
        # padding -> quarter-local dummy row (all-zero, core 2q's last row)
        idxs = np.empty(SLOTS, np.int16)
        q_of_slot = blk_q[np.arange(SLOTS) // KBLK]
        idxs[:] = (NPAD - 1)  # dummy local idx within any quarter
        idxs_local_dummy = np.full(SLOTS, NPAD - 1, np.int64)
        idxs_local = idxs_local_dummy.copy()
        idxs_local[slot] = e_gr - (e_gr // QROWS) * QROWS
        assert idxs_local.max() < 32768
        col_slots = np.zeros(SLOTS, np.float16)
        ew_slots = np.zeros(SLOTS, np.float16)
        col_slots[slot] = (e_cl % SW).astype(np.float16)
        ew_slots[slot] = e_w
        # dma_gather idx layout: stream i reads idx[i%16, i//16], 16-partition
        # wrapped, replicated across the 8 q7 core groups
        i16 = idxs_local.astype(np.int16).reshape(-1, 16).T  # [16, SLOTS/16]
        a_idx16 = np.ascontiguousarray(np.tile(i16, (8, 1)))  # [128, SLOTS/16]
        a_col = np.ascontiguousarray(col_slots.reshape(TOTBLK, KBLK).T)
        a_ew = np.ascontiguousarray(ew_slots.reshape(TOTBLK, KBLK).T)
        csr_l = np.ascontiguousarray(
            csr[k * NPAD : (k + 1) * NPAD].reshape(NWINL, WIN, K1).transpose(0, 2, 1)
        )
        core_arrays.append(dict(aidx=a_idx16, acol=a_col, aew=a_ew, csrl=csr_l))

    x16 = np.zeros((NTOT, D), np.float16)
    allg = (np.minimum(loop // NLOC, G - 1) * NPAD
            + (loop - np.minimum(loop // NLOC, G - 1) * NLOC)).astype(np.int64)
    x16[allg] = x.astype(np.float16)

    cfg = dict(
        N=N, NLOC=NLOC, NPAD=NPAD, NTOT=NTOT, QROWS=QROWS, NWINL=NWINL,
        NWINF=NWINF, NSW=NSW, K1=K1, TOTBLK=TOTBLK,
        blk_sw=blk_sw, block_first=block_first, block_last=block_last,
        batches=batches, win_blocks=win_blocks,
    )
    return cfg, core_arrays, x16, csr_pt


def build_program(cfg):
    import concourse.bacc as bacc
    import concourse.mybir as mybir
    import concourse.tile as tile

    f32 = mybir.dt.float32
    f16 = mybir.dt.float16
    i16 = mybir.dt.int16
    ALU = mybir.AluOpType
    AF = mybir.ActivationFunctionType

    NTOT, NPAD, QROWS = cfg["NTOT"], cfg["NPAD"], cfg["QROWS"]
    NWINL, NWINF, K1 = cfg["NWINL"], cfg["NWINF"], cfg["K1"]
    TOTBLK = cfg["TOTBLK"]
    blk_sw = cfg["blk_sw"]
    block_first = cfg["block_first"]
    block_last = cfg["block_last"]
    batches = cfg["batches"]
    win_blocks = cfg["win_blocks"]

    nc = bacc.Bacc("TRN2", num_devices=G)

    t_x16 = nc.dram_tensor("x16", [NTOT, D], f16, kind="ExternalInput")
    t_csrf = nc.dram_tensor("csrf", [NWINF, K1, WIN], f16, kind="ExternalInput")
    t_csrl = nc.dram_tensor("csrl", [NWINL, K1, WIN], f16, kind="ExternalInput")
    t_aidx = nc.dram_tensor("aidx", [128, TOTBLK * 8], i16, kind="ExternalInput")
    t_acol = nc.dram_tensor("acol", [KBLK, TOTBLK], f16, kind="ExternalInput")
    t_aew = nc.dram_tensor("aew", [KBLK, TOTBLK], f16, kind="ExternalInput")
    t_iota = nc.dram_tensor("iota", [128, SW], f16, kind="ExternalInput")
    t_ident = nc.dram_tensor("ident", [128, 128], f32, kind="ExternalInput")
    t_w1 = nc.dram_tensor("w1f", [D, H], f32, kind="ExternalInput")
    t_w2h = nc.dram_tensor("w2h", [H, H], f16, kind="ExternalInput")
    t_a1h = nc.dram_tensor("a1h", [H, H], f16, kind="ExternalInput")
    t_b1f = nc.dram_tensor("b1f", [H, 1], f32, kind="ExternalInput")
    t_a2h = nc.dram_tensor("a2h", [H, OUTF], f16, kind="ExternalInput")
    t_b2r = nc.dram_tensor("b2r", [128, OUTF], f32, kind="ExternalInput")

    # gather tables: fp16 rows padded to DP columns (256B elements).
    # right halves are never written nor read.
    t_xp = nc.dram_tensor("xp", [NTOT, DP], f16)
    t_z2t = nc.dram_tensor("z2t", [NTOT, DP], f16, addr_space="Shared")
    t_z2s = nc.dram_tensor("z2s", [NPAD, DP], f16)

    t_h2o = nc.dram_tensor("h2o", [NPAD, H], f32, kind="ExternalOutput")
    t_ro = nc.dram_tensor("ro", [NPAD, OUTF], f32, kind="ExternalOutput")

    with tile.TileContext(nc) as tc, ExitStack() as ctx:
        const = ctx.enter_context(tc.tile_pool(name="const", bufs=1))
        persist = ctx.enter_context(tc.tile_pool(name="persist", bufs=1))
        edgep = ctx.enter_context(tc.tile_pool(name="edgep", bufs=1))
        csr_p = ctx.enter_context(tc.tile_pool(name="csrp", bufs=2))
        xp_p = ctx.enter_context(tc.tile_pool(name="xpp", bufs=3))
        idx_p = ctx.enter_context(tc.tile_pool(name="idxp", bufs=3))
        msg_p = ctx.enter_context(tc.tile_pool(name="msgp", bufs=2))
        s_p = ctx.enter_context(tc.tile_pool(name="sp", bufs=2))
        ep_p = ctx.enter_context(tc.tile_pool(name="epp", bufs=2))
        ps_agg = ctx.enter_context(tc.tile_pool(name="psagg", bufs=2, space="PSUM"))
        ps_eps = ctx.enter_context(tc.tile_pool(name="pseps", bufs=1, space="PSUM"))

        def _load(pool, t, shape, dtype):
            tl = pool.tile(shape, dtype, tag=t.name + "_sb")
            nc.sync.dma_start(out=tl[:], in_=t[:, :])
            return tl

        sb_iota = _load(const, t_iota, [128, SW], f16)
        sb_ident = _load(const, t_ident, [128, 128], f32)
        sb_w1 = _load(const, t_w1, [D, H], f32)
        sb_w2h = _load(const, t_w2h, [H, H], f16)
        sb_a1h = _load(const, t_a1h, [H, H], f16)
        sb_b1f = _load(const, t_b1f, [H, 1], f32)
        sb_a2h = _load(const, t_a2h, [H, OUTF], f16)
        sb_b2r = _load(const, t_b2r, [128, OUTF], f32)
        sb_acol = edgep.tile([KBLK, TOTBLK], f16, tag="acol_sb")
        nc.sync.dma_start(out=sb_acol[:], in_=t_acol[:, :])
        sb_aew = edgep.tile([KBLK, TOTBLK], f16, tag="aew_sb")
        nc.sync.dma_start(out=sb_aew[:], in_=t_aew[:, :])

        # ---- degrees -> dinv
        deg_f = persist.tile([128, NWINF], f32, tag="degf")
        deg_l = persist.tile([128, NWINL], f32, tag="degl")
        dinv_f16 = persist.tile([128, NWINF], f16, tag="dinvf16")
        dinv_l = persist.tile([128, NWINL], f32, tag="dinvl")

        def deg_phase(t_csr, nwin, out_deg):
            for b0 in range(0, nwin, NB_DEG):
                nb = min(NB_DEG, nwin - b0)
                tl = csr_p.tile([128, NB_DEG * K1], f16, tag="csr_t")
                src = t_csr[b0 : b0 + nb, :, :].rearrange("b k p -> p b k")
                dst3 = tl[:].rearrange("p (b k) -> p b k", k=K1)
                nc.sync.dma_start(out=dst3[:, :nb, :], in_=src)
                nc.vector.tensor_reduce(
                    out=out_deg[:, b0 : b0 + nb], in_=dst3[:, :nb, :],
                    axis=mybir.AxisListType.X, op=ALU.add,
                )

        deg_phase(t_csrf, NWINF, deg_f)
        deg_phase(t_csrl, NWINL, deg_l)

        rec_f = persist.tile([128, NWINF], f32, tag="recf")
        nc.vector.reciprocal(out=rec_f[:], in_=deg_f[:])
        dinv_f = persist.tile([128, NWINF], f32, tag="dinvf")
        nc.scalar.sqrt(out=dinv_f[:], in_=rec_f[:])
        nc.vector.tensor_copy(out=dinv_f16[:], in_=dinv_f[:])
        rec_l = persist.tile([128, NWINL], f32, tag="recl")
        nc.vector.reciprocal(out=rec_l[:], in_=deg_l[:])
        nc.scalar.sqrt(out=dinv_l[:], in_=rec_l[:])

        # ---- xp = dinv * x16 (full padded table, left halves only)
        for b0 in range(0, NWINF, NB_XP):
            nb = min(NB_XP, NWINF - b0)
            xt = xp_p.tile([128, NB_XP * D], f16, tag="xt")
            src = t_x16[b0 * WIN : (b0 + nb) * WIN, :].rearrange(
                "(b p) d -> p b d", p=WIN)
            xt3 = xt[:].rearrange("p (b d) -> p b d", d=D)
            nc.sync.dma_start(out=xt3[:, :nb, :], in_=src)
            xs = xp_p.tile([128, NB_XP * D], f16, tag="xs")
            xs3 = xs[:].rearrange("p (b d) -> p b d", d=D)
            dv = dinv_f16[:, b0 : b0 + nb].unsqueeze(2).to_broadcast([128, nb, D])
            nc.vector.tensor_tensor(
                out=xs3[:, :nb, :], in0=xt3[:, :nb, :], in1=dv, op=ALU.mult)
            dst = t_xp[b0 * WIN : (b0 + nb) * WIN, 0:D].rearrange(
                "(b p) d -> p b d", p=WIN)
            nc.sync.dma_start(out=dst, in_=xs3[:, :nb, :])

        max_batch_blk = max(bt["bhi"] - bt["blo"] for bt in batches)
        max_win_blk = max(len(bl) for bl in win_blocks)

        # ---- one aggregation layer over a padded table
        def layer(table_t, epilogue):
            for bt in batches:
                g0, g1 = bt["g0"], bt["g1"]
                blo, bhi = bt["blo"], bt["bhi"]
                nb = bhi - blo
                idx_sb = idx_p.tile([128, max_batch_blk * 8], i16, tag="idxt")
                nc.sync.dma_start(
                    out=idx_sb[:, : nb * 8], in_=t_aidx[:, blo * 8 : bhi * 8])
                msg = msg_p.tile([128, max_batch_blk * DP], f16, tag="msg")
                msg3 = msg[:].rearrange("p (b d) -> p b d", d=DP)
                for q, (qlo, qhi) in enumerate(bt["q_ranges"]):
                    # dma_gather misbehaves above ~1024 idxs/call; chunk it
                    for c0 in range(qlo, qhi, 8):
                        c1 = min(c0 + 8, qhi)
                        ncb = c1 - c0
                        nc.gpsimd.dma_gather(
                            out_ap=msg3[:, c0 - blo : c1 - blo, :],
                            in_ap=table_t[q * QROWS : (q + 1) * QROWS, :],
                            idxs_ap=idx_sb[:, (c0 - blo) * 8 : (c1 - blo) * 8],
                            num_idxs=ncb * KBLK,
                            num_idxs_reg=ncb * KBLK,
                            elem_size=DP,
                        )
                st = s_p.tile([128, max_batch_blk * SW], f16, tag="stile")
                st3 = st[:].rearrange("p (b t) -> p b t", t=SW)
                io_b = sb_iota[:, :].unsqueeze(1).to_broadcast([128, nb, SW])
                cl_b = sb_acol[:, blo:bhi].unsqueeze(2).to_broadcast([128, nb, SW])
                ew_b = sb_aew[:, blo:bhi].unsqueeze(2).to_broadcast([128, nb, SW])
                parts = int(os.environ.get("GCN_L1PARTS", "3"))
                if parts < 2:
                    continue
                nc.vector.tensor_tensor(
                    out=st3[:, :nb, :], in0=io_b, in1=cl_b, op=ALU.is_equal)
                nc.vector.tensor_tensor(
                    out=st3[:, :nb, :], in0=st3[:, :nb, :], in1=ew_b, op=ALU.mult)
                if parts < 3:
                    continue
                for w in range(g0, g1):
                    psa = ps_agg.tile([128, D], f32, tag="psa")
                    for b in win_blocks[w]:
                        j = int(blk_sw[b]) % 4
                        # skip_group_check: sim's psum zero-region bookkeeping
                        # mis-addresses partition-sliced outputs
                        nc.tensor.matmul(
                            out=psa[j * SW : (j + 1) * SW, :],
                            lhsT=st3[:, b - blo, :],
                            rhs=msg3[:, b - blo, 0:D],
                            start=bool(block_first[b]),
                            stop=bool(block_last[b]),
                            tile_position=(0, j * SW),
                            skip_group_check=True,
                        )
                    epilogue(w, psa)

        # ---- layer 1 epilogue: z2s_w = dinv * (relu(dinv*agg @ W1) @ W2)
        def epi1(w, psa):
            agg = ep_p.tile([128, D], f32, tag="agg")
            nc.vector.tensor_scalar(
                out=agg[:], in0=psa[:], scalar1=dinv_l[:, w : w + 1], scalar2=None,
                op0=ALU.mult)
            pst = ps_eps.tile([64, 128], f32, tag="pst")
            nc.tensor.transpose(out=pst[:], in_=agg[:], identity=sb_ident[:])
            aggT = ep_p.tile([64, 128], f32, tag="aggT")
            nc.scalar.activation(out=aggT[:], in_=pst[:], func=AF.Copy)
            psh = ps_eps.tile([64, 128], f32, tag="psh")
            nc.tensor.matmul(out=psh[:], lhsT=sb_w1[:], rhs=aggT[:], start=True, stop=True)
            h1T = ep_p.tile([64, 128], f16, tag="h1T")
            nc.scalar.activation(out=h1T[:], in_=psh[:], func=AF.Relu)
            psz = ps_eps.tile([128, D], f32, tag="psz")
            nc.tensor.matmul(out=psz[:], lhsT=h1T[:], rhs=sb_w2h[:], start=True, stop=True)
            z2 = ep_p.tile([128, D], f16, tag="z2")
            nc.vector.tensor_scalar(
                out=z2[:], in0=psz[:], scalar1=dinv_l[:, w : w + 1], scalar2=None,
                op0=ALU.mult)
            nc.sync.dma_start(out=t_z2s[w * WIN : (w + 1) * WIN, 0:D], in_=z2[:])

        stage = int(os.environ.get("GCN_STAGE", "3"))
        if stage >= 1:
            layer(t_xp, epi1)
        if stage >= 2:
            nc.gpsimd.collective_compute(
                "AllGather",
                mybir.AluOpType.bypass,
                ins=[t_z2s[:, :]],
                outs=[t_z2t[:, :]],
                replica_groups=[list(range(G))],
            )

        # ---- layer 2 epilogue: h2 = relu(dinv*agg2); readout MLP
        def epi2(w, psa):
            h2r = ep_p.tile([128, H], f32, tag="h2r")
            nc.vector.tensor_scalar(
                out=h2r[:], in0=psa[:], scalar1=dinv_l[:, w : w + 1], scalar2=0.0,
                op0=ALU.mult, op1=ALU.max)
            nc.sync.dma_start(out=t_h2o[w * WIN : (w + 1) * WIN, :], in_=h2r[:])
            pst = ps_eps.tile([64, 128], f32, tag="pst")
            nc.tensor.transpose(out=pst[:], in_=h2r[:], identity=sb_ident[:])
            h2T = ep_p.tile([64, 128], f16, tag="h1T")
            nc.scalar.activation(out=h2T[:], in_=pst[:], func=AF.Copy)
            pst1 = ps_eps.tile([64, 128], f32, tag="pst1")
            nc.tensor.matmul(out=pst1[:], lhsT=sb_a1h[:], rhs=h2T[:], start=True, stop=True)
            t1T = ep_p.tile([64, 128], f16, tag="t1T")
            nc.scalar.activation(
                out=t1T[:], in_=pst1[:], func=AF.Relu, bias=sb_b1f[:, 0:1])
            psr = ps_eps.tile([128, OUTF], f32, tag="psr")
            nc.tensor.matmul(out=psr[:], lhsT=t1T[:], rhs=sb_a2h[:], start=True, stop=True)
            rs = ep_p.tile([128, OUTF], f32, tag="rs")
            nc.vector.tensor_tensor(out=rs[:], in0=psr[:], in1=sb_b2r[:], op=ALU.add)
            nc.sync.dma_start(out=t_ro[w * WIN : (w + 1) * WIN, :], in_=rs[:])

        if stage >= 3:
            layer(t_z2t, epi2)

    nc.compile()
    return nc


def make_in_maps(cfg, core_arrays, x16, csr_pt, weights):
    W1, W2, A1, b1, A2, b2 = [np.asarray(w, np.float32) for w in weights]
    shared = dict(
        x16=x16,
        csrf=csr_pt,
        iota=np.tile(np.arange(SW, dtype=np.float16), (128, 1)),
        ident=np.eye(128, dtype=np.float32),
        w1f=W1,
        w2h=W2.astype(np.float16),
        a1h=A1.astype(np.float16),
        b1f=b1.reshape(H, 1),
        a2h=A2.astype(np.float16),
        b2r=np.tile(b2.reshape(1, OUTF), (128, 1)),
    )
    in_maps = []
    for k in range(G):
        m = dict(shared)
        ca = core_arrays[k]
        m["csrl"] = ca["csrl"]
        m["aidx"] = ca["aidx"]
        m["acol"] = ca["acol"]
        m["aew"] = ca["aew"]
        in_maps.append(m)
    return in_maps


_LAST_RESULTS = {}
_PROG_CACHE = {}


def kernel(x, edge_index, edge_weight, W1, W2, A1, b1, A2, b2):
    x = np.asarray(x, np.float32)
    N = x.shape[0]
    cfg, core_arrays, x16, csr_pt = host_prep(x, edge_index, edge_weight)
    key = (N, cfg["TOTBLK"], cfg["K1"], bytes(np.asarray(cfg["blk_sw"]).data))
    nc = _PROG_CACHE.get(key)
    if nc is None:
        nc = build_program(cfg)
        _PROG_CACHE.clear()
        _PROG_CACHE[key] = nc
    in_maps = make_in_maps(cfg, core_arrays, x16, csr_pt, (W1, W2, A1, b1, A2, b2))

    from concourse import bass_utils

    trace = bool(os.environ.get("GCN_TRACE"))
    res = bass_utils.run_bass_kernel_spmd(
        nc, in_maps, core_ids=list(range(G)), trace=trace
    )
    _LAST_RESULTS["exec_time_ns"] = res.exec_time_ns
    _LAST_RESULTS["results"] = res

    NLOC = cfg["NLOC"]
    h_parts, r_parts = [], []
    for k in range(G):
        n_real = min(NLOC, max(0, N - k * NLOC))
        h_parts.append(res.results[k]["h2o"][:n_real])
        r_parts.append(res.results[k]["ro"][:n_real])
    h = np.concatenate(h_parts, axis=0).astype(np.float32)
    r = np.concatenate(r_parts, axis=0).reshape(N, HOR, OUT).astype(np.float32)
    return (r, h)


# revision 28
# speedup vs baseline: 1.3727x; 1.3727x over previous
"""GCN decoder (2x GCNConv + MLP readout) on 8 Trainium2 NeuronCores.

Sharding: nodes are partitioned across the 8 cores by target-node range
(graph parallel). Edges are routed (on host) to the core that owns their
target node and sorted by target. Each core aggregates messages for its
own nodes with dma_gather + selection-matrix matmuls (scatter-add becomes
dense PE work); the layer-2 feature table is exchanged with one AllGather.

dma_gather uses int16 indices over 256B elements, so the feature tables
are stored as fp16 rows padded to 128 columns (256B) and edges are grouped
into blocks whose sources all come from one quarter of the table
(quarter size 25088 < int16 range).

self-contained: hardcodes the problem geometry, imports only installed
packages (numpy / concourse).
"""

import os
from contextlib import ExitStack

import numpy as np

# ---------------- fixed geometry ----------------
G = 8          # cores
NQ = 4         # table quarters (int16 index range for dma_gather)
WIN = 128      # nodes per PSUM accumulation window
SW = 32        # nodes per matmul output slice (sub-window)
KBLK = 128     # edge slots per matmul block
D = 64         # input feature dim
DP = 128       # padded row width (256B rows for dma_gather)
H = 64         # hidden dim
OUT = 32
HOR = 12
OUTF = OUT * HOR

# batching knobs
NB_DEG = 8     # windows per deg-reduce batch
NB_XP = 8      # windows per x-prescale batch
WG = 4         # windows per gather batch


def _cdiv(a, b):
    return (a + b - 1) // b


def _group_positions(sorted_group_ids, n_groups):
    counts = np.bincount(sorted_group_ids, minlength=n_groups)
    offsets = np.concatenate([[0], np.cumsum(counts)[:-1]])
    return np.arange(len(sorted_group_ids)) - offsets[sorted_group_ids], counts, offsets


def host_prep(x, edge_index, edge_weight):
    """Route/sort/pad edges, build CSR + slot arrays + static block schedule."""
    x = np.asarray(x, np.float32)
    edge_index = np.asarray(edge_index).astype(np.int64)
    edge_weight = np.asarray(edge_weight, np.float32)

    N = x.shape[0]
    NLOC = _cdiv(N, G)
    NPAD = _cdiv(NLOC, WIN) * WIN
    NTOT = G * NPAD
    QROWS = NTOT // NQ
    NWINL = NPAD // WIN
    NWINF = NTOT // WIN
    NSW = NPAD // SW
    assert NLOC < NPAD, "need dummy rows for gather padding"
    assert QROWS < 32768, "quarter must fit int16 index range"

    loop = np.arange(N, dtype=np.int64)
    row = np.concatenate([edge_index[0], loop])
    col = np.concatenate([edge_index[1], loop])
    ew = np.concatenate([edge_weight, np.ones(N, np.float32)])

    core_of = np.minimum(col // NLOC, G - 1)
    rcore = np.minimum(row // NLOC, G - 1)
    gid_row = (rcore * NPAD + (row - rcore * NLOC)).astype(np.int64)

    # ---- full-graph CSR of edge weights, gid order, for on-device degrees
    gcol = (core_of * NPAD + (col - core_of * NLOC)).astype(np.int64)
    order = np.argsort(gcol, kind="stable")
    gcol_s = gcol[order]
    pos, counts_all, _ = _group_positions(gcol_s, NTOT)
    K1 = max(8, int(counts_all.max()))
    K1 = _cdiv(K1, 8) * 8
    csr = np.zeros((NTOT, K1), np.float16)
    csr[gcol_s, pos] = ew[order].astype(np.float16)
    dummy_mask = np.zeros(NTOT, bool)
    for k in range(G):
        n_real = min(NLOC, max(0, N - k * NLOC))
        dummy_mask[k * NPAD + n_real : (k + 1) * NPAD] = True
    csr[dummy_mask, 0] = 1.0
    csr_pt = np.ascontiguousarray(csr.reshape(NWINF, WIN, K1).transpose(0, 2, 1))

    # ---- per-core edges sorted by (target sub-window, source quarter)
    per_core = []
    counts_swq = np.zeros((G, NSW * NQ), np.int64)
    for k in range(G):
        m = core_of == k
        e_gr = gid_row[m]
        e_cl = (col[m] - k * NLOC).astype(np.int64)
        e_w = ew[m].astype(np.float16)
        e_q = e_gr // QROWS
        key = (e_cl // SW) * NQ + e_q
        o = np.argsort(key, kind="stable")
        e_gr, e_cl, e_w, key = e_gr[o], e_cl[o], e_w[o], key[o]
        counts_swq[k] = np.bincount(key, minlength=NSW * NQ)
        per_core.append((e_gr, e_cl, e_w, key))

    # blocks per (sw, q): shared static schedule; q of each block is static
    Bq = _cdiv(counts_swq.max(axis=0), KBLK).reshape(NSW, NQ)  # may be 0
    # every sub-window needs >=1 block so its psum slice gets initialized;
    # self-loops guarantee the own-core quarter is nonempty, but be safe:
    for swi in range(NSW):
        if Bq[swi].sum() == 0:
            Bq[swi, 0] = 1

    # global block order: per gather batch (WG windows), quarter-major
    batches = []
    blk_sw = []
    blk_q = []
    blk_of_swq = {}
    for g0 in range(0, NWINL, WG):
        g1 = min(g0 + WG, NWINL)
        b_start = len(blk_sw)
        q_ranges = []
        for q in range(NQ):
            q_lo = len(blk_sw)
            for w in range(g0, g1):
                for swi in range(w * 4, w * 4 + 4):
                    nbq = int(Bq[swi, q])
                    if nbq:
                        blk_of_swq[(swi, q)] = (len(blk_sw), nbq)
                        for _ in range(nbq):
                            blk_sw.append(swi)
                            blk_q.append(q)
            q_ranges.append((q_lo, len(blk_sw)))
        batches.append(dict(g0=g0, g1=g1, blo=b_start, bhi=len(blk_sw),
                            q_ranges=q_ranges))
    TOTBLK = len(blk_sw)
    SLOTS = TOTBLK * KBLK
    blk_sw = np.asarray(blk_sw)
    blk_q = np.asarray(blk_q)
    # first/last block of each sub-window in global order
    block_first = np.zeros(TOTBLK, bool)
    block_last = np.zeros(TOTBLK, bool)
    seen = set()
    last_of = {}
    for b in range(TOTBLK):
        swi = int(blk_sw[b])
        if swi not in seen:
            seen.add(swi)
            block_first[b] = True
        last_of[swi] = b
    for b in last_of.values():
        block_last[b] = True
    win_blocks = [[] for _ in range(NWINL)]
    for b in range(TOTBLK):
        win_blocks[int(blk_sw[b]) // 4].append(b)

    core_arrays = []
    for k in range(G):
        e_gr, e_cl, e_w, key = per_core[k]
        # slot position for each edge: its (sw, q) block region + within-pos
        wpos, _, _ = _group_positions(key, NSW * NQ)
        base = np.zeros(NSW * NQ, np.int64)
        for (swi, q), (b0, nbq) in blk_of_swq.items():
            base[swi * NQ + q] = b0 * KBLK
        slot = base[key] + wpos
        # padding -> quarter-local dummy row (all-zero, core 2q's last row)
        idxs = np.empty(SLOTS, np.int16)
        q_of_slot = blk_q[np.arange(SLOTS) // KBLK]
        idxs[:] = (NPAD - 1)  # dummy local idx within any quarter
        idxs_local_dummy = np.full(SLOTS, NPAD - 1, np.int64)
        idxs_local = idxs_local_dummy.copy()
        idxs_local[slot] = e_gr - (e_gr // QROWS) * QROWS
        assert idxs_local.max() < 32768
        col_slots = np.zeros(SLOTS, np.float16)
        ew_slots = np.zeros(SLOTS, np.float16)
        col_slots[slot] = (e_cl % SW).astype(np.float16)
        ew_slots[slot] = e_w
        # dma_gather idx layout: stream i reads idx[i%16, i//16], 16-partition
        # wrapped, replicated across the 8 q7 core groups
        a_idx16 = np.ascontiguousarray(
            idxs_local.astype(np.int16).reshape(-1, 16).T)  # [16, SLOTS/16]
        a_col = np.ascontiguousarray(col_slots.reshape(TOTBLK, KBLK).T)
        a_ew = np.ascontiguousarray(ew_slots.reshape(TOTBLK, KBLK).T)
        csr_l = np.ascontiguousarray(
            csr[k * NPAD : (k + 1) * NPAD].reshape(NWINL, WIN, K1).transpose(0, 2, 1)
        )
        core_arrays.append(dict(aidx=a_idx16, acol=a_col, aew=a_ew, csrl=csr_l))

    x16 = np.zeros((NTOT, D), np.float16)
    allg = (np.minimum(loop // NLOC, G - 1) * NPAD
            + (loop - np.minimum(loop // NLOC, G - 1) * NLOC)).astype(np.int64)
    x16[allg] = x.astype(np.float16)

    cfg = dict(
        N=N, NLOC=NLOC, NPAD=NPAD, NTOT=NTOT, QROWS=QROWS, NWINL=NWINL,
        NWINF=NWINF, NSW=NSW, K1=K1, TOTBLK=TOTBLK,
        blk_sw=blk_sw, block_first=block_first, block_last=block_last,
        batches=batches, win_blocks=win_blocks,
    )
    return cfg, core_arrays, x16, csr_pt


def build_program(cfg):
    import concourse.bacc as bacc
    import concourse.mybir as mybir
    import concourse.tile as tile

    f32 = mybir.dt.float32
    f16 = mybir.dt.float16
    i16 = mybir.dt.int16
    ALU = mybir.AluOpType
    AF = mybir.ActivationFunctionType

    NTOT, NPAD, QROWS = cfg["NTOT"], cfg["NPAD"], cfg["QROWS"]
    NWINL, NWINF, K1 = cfg["NWINL"], cfg["NWINF"], cfg["K1"]
    TOTBLK = cfg["TOTBLK"]
    blk_sw = cfg["blk_sw"]
    block_first = cfg["block_first"]
    block_last = cfg["block_last"]
    batches = cfg["batches"]
    win_blocks = cfg["win_blocks"]

    nc = bacc.Bacc("TRN2", num_devices=G)

    t_x16 = nc.dram_tensor("x16", [NTOT, D], f16, kind="ExternalInput")
    t_csrf = nc.dram_tensor("csrf", [NWINF, K1, WIN], f16, kind="ExternalInput")
    t_csrl = nc.dram_tensor("csrl", [NWINL, K1, WIN], f16, kind="ExternalInput")
    t_aidx = nc.dram_tensor("aidx", [16, TOTBLK * 8], i16, kind="ExternalInput")
    t_acol = nc.dram_tensor("acol", [KBLK, TOTBLK], f16, kind="ExternalInput")
    t_aew = nc.dram_tensor("aew", [KBLK, TOTBLK], f16, kind="ExternalInput")
    t_iota = nc.dram_tensor("iota", [128, SW], f16, kind="ExternalInput")
    t_ident = nc.dram_tensor("ident", [128, 128], f32, kind="ExternalInput")
    t_ident16 = nc.dram_tensor("ident16", [128, 128], f16, kind="ExternalInput")
    t_w1 = nc.dram_tensor("w1f", [D, H], f32, kind="ExternalInput")
    t_w2h = nc.dram_tensor("w2h", [H, H], f16, kind="ExternalInput")
    t_a1h = nc.dram_tensor("a1h", [H, H], f16, kind="ExternalInput")
    t_b1f = nc.dram_tensor("b1f", [H, 1], f32, kind="ExternalInput")
    t_a2h = nc.dram_tensor("a2h", [H, OUTF], f16, kind="ExternalInput")
    t_b2r = nc.dram_tensor("b2r", [128, OUTF], f32, kind="ExternalInput")

    # gather tables: fp16 rows padded to DP columns (256B elements).
    # right halves are never written nor read.
    t_xp = nc.dram_tensor("xp", [NTOT, DP], f16)
    t_z2t = nc.dram_tensor("z2t", [NTOT, DP], f16, addr_space="Shared")
    t_z2s = nc.dram_tensor("z2s", [NPAD, DP], f16)

    t_h2o = nc.dram_tensor("h2o", [NPAD, H], f16, kind="ExternalOutput")
    t_ro = nc.dram_tensor("ro", [NPAD, OUTF], f16, kind="ExternalOutput")

    with tile.TileContext(nc) as tc, ExitStack() as ctx:
        const = ctx.enter_context(tc.tile_pool(name="const", bufs=1))
        persist = ctx.enter_context(tc.tile_pool(name="persist", bufs=1))
        edgep = ctx.enter_context(tc.tile_pool(name="edgep", bufs=1))
        csr_p = ctx.enter_context(tc.tile_pool(name="csrp", bufs=2))
        xp_p = ctx.enter_context(tc.tile_pool(name="xpp", bufs=3))
        idx_p = ctx.enter_context(tc.tile_pool(name="idxp", bufs=3))
        msg_p = ctx.enter_context(tc.tile_pool(name="msgp", bufs=2))
        s_p = ctx.enter_context(tc.tile_pool(name="sp", bufs=2))
        ep_p = ctx.enter_context(tc.tile_pool(name="epp", bufs=2))
        ps_agg = ctx.enter_context(tc.tile_pool(name="psagg", bufs=2, space="PSUM"))
        ps_eps = ctx.enter_context(tc.tile_pool(name="pseps", bufs=1, space="PSUM"))

        def _load(pool, t, shape, dtype):
            tl = pool.tile(shape, dtype, tag=t.name + "_sb")
            nc.sync.dma_start(out=tl[:], in_=t[:, :])
            return tl

        sb_iota = _load(const, t_iota, [128, SW], f16)
        sb_ident = _load(const, t_ident, [128, 128], f32)
        sb_ident16 = _load(const, t_ident16, [128, 128], f16)
        sb_w1 = _load(const, t_w1, [D, H], f32)
        sb_w2h = _load(const, t_w2h, [H, H], f16)
        sb_a1h = _load(const, t_a1h, [H, H], f16)
        sb_b1f = _load(const, t_b1f, [H, 1], f32)
        sb_a2h = _load(const, t_a2h, [H, OUTF], f16)
        sb_b2r = _load(const, t_b2r, [128, OUTF], f32)
        sb_acol = edgep.tile([KBLK, TOTBLK], f16, tag="acol_sb")
        nc.sync.dma_start(out=sb_acol[:], in_=t_acol[:, :])
        sb_aew = edgep.tile([KBLK, TOTBLK], f16, tag="aew_sb")
        nc.sync.dma_start(out=sb_aew[:], in_=t_aew[:, :])

        # ---- degrees -> dinv
        deg_f = persist.tile([128, NWINF], f32, tag="degf")
        deg_l = persist.tile([128, NWINL], f32, tag="degl")
        dinv_f16 = persist.tile([128, NWINF], f16, tag="dinvf16")
        dinv_l = persist.tile([128, NWINL], f32, tag="dinvl")

        def deg_phase(t_csr, nwin, out_deg):
            for b0 in range(0, nwin, NB_DEG):
                nb = min(NB_DEG, nwin - b0)
                tl = csr_p.tile([128, NB_DEG * K1], f16, tag="csr_t")
                src = t_csr[b0 : b0 + nb, :, :].rearrange("b k p -> p b k")
                dst3 = tl[:].rearrange("p (b k) -> p b k", k=K1)
                nc.sync.dma_start(out=dst3[:, :nb, :], in_=src)
                nc.vector.tensor_reduce(
                    out=out_deg[:, b0 : b0 + nb], in_=dst3[:, :nb, :],
                    axis=mybir.AxisListType.X, op=ALU.add,
                )

        deg_phase(t_csrf, NWINF, deg_f)
        deg_phase(t_csrl, NWINL, deg_l)

        rec_f = persist.tile([128, NWINF], f32, tag="recf")
        nc.vector.reciprocal(out=rec_f[:], in_=deg_f[:])
        dinv_f = persist.tile([128, NWINF], f32, tag="dinvf")
        nc.scalar.sqrt(out=dinv_f[:], in_=rec_f[:])
        nc.vector.tensor_copy(out=dinv_f16[:], in_=dinv_f[:])
        rec_l = persist.tile([128, NWINL], f32, tag="recl")
        nc.vector.reciprocal(out=rec_l[:], in_=deg_l[:])
        nc.scalar.sqrt(out=dinv_l[:], in_=rec_l[:])

        # ---- xp = dinv * x16 (full padded table, left halves only)
        for b0 in range(0, NWINF, NB_XP):
            nb = min(NB_XP, NWINF - b0)
            xt = xp_p.tile([128, NB_XP * D], f16, tag="xt")
            src = t_x16[b0 * WIN : (b0 + nb) * WIN, :].rearrange(
                "(b p) d -> p b d", p=WIN)
            xt3 = xt[:].rearrange("p (b d) -> p b d", d=D)
            nc.sync.dma_start(out=xt3[:, :nb, :], in_=src)
            xs = xp_p.tile([128, NB_XP * D], f16, tag="xs")
            xs3 = xs[:].rearrange("p (b d) -> p b d", d=D)
            dv = dinv_f16[:, b0 : b0 + nb].unsqueeze(2).to_broadcast([128, nb, D])
            nc.vector.tensor_tensor(
                out=xs3[:, :nb, :], in0=xt3[:, :nb, :], in1=dv, op=ALU.mult)
            dst = t_xp[b0 * WIN : (b0 + nb) * WIN, 0:D].rearrange(
                "(b p) d -> p b d", p=WIN)
            nc.sync.dma_start(out=dst, in_=xs3[:, :nb, :])

        max_batch_blk = max(bt["bhi"] - bt["blo"] for bt in batches)
        max_win_blk = max(len(bl) for bl in win_blocks)

        # ---- one aggregation layer over a padded table
        def layer(table_t, epilogue):
            for bt in batches:
                g0, g1 = bt["g0"], bt["g1"]
                blo, bhi = bt["blo"], bt["bhi"]
                nb = bhi - blo
                idx_sb = idx_p.tile([128, max_batch_blk * 8], i16, tag="idxt")
                for gk in range(8):
                    nc.sync.dma_start(
                        out=idx_sb[gk * 16 : (gk + 1) * 16, : nb * 8],
                        in_=t_aidx[:, blo * 8 : bhi * 8])
                msg = msg_p.tile([128, max_batch_blk * DP], f16, tag="msg")
                msg3 = msg[:].rearrange("p (b d) -> p b d", d=DP)
                for q, (qlo, qhi) in enumerate(bt["q_ranges"]):
                    # dma_gather misbehaves above ~1024 idxs/call; chunk it
                    for c0 in range(qlo, qhi, 8):
                        c1 = min(c0 + 8, qhi)
                        ncb = c1 - c0
                        nc.gpsimd.dma_gather(
                            out_ap=msg3[:, c0 - blo : c1 - blo, :],
                            in_ap=table_t[q * QROWS : (q + 1) * QROWS, :],
                            idxs_ap=idx_sb[:, (c0 - blo) * 8 : (c1 - blo) * 8],
                            num_idxs=ncb * KBLK,
                            num_idxs_reg=ncb * KBLK,
                            elem_size=DP,
                        )
                st = s_p.tile([128, max_batch_blk * SW], f16, tag="stile")
                st3 = st[:].rearrange("p (b t) -> p b t", t=SW)
                io_b = sb_iota[:, :].unsqueeze(1).to_broadcast([128, nb, SW])
                cl_b = sb_acol[:, blo:bhi].unsqueeze(2).to_broadcast([128, nb, SW])
                ew_b = sb_aew[:, blo:bhi].unsqueeze(2).to_broadcast([128, nb, SW])
                parts = int(os.environ.get("GCN_L1PARTS", "3"))
                if parts < 2:
                    continue
                nc.vector.tensor_tensor(
                    out=st3[:, :nb, :], in0=io_b, in1=cl_b, op=ALU.is_equal)
                nc.vector.tensor_tensor(
                    out=st3[:, :nb, :], in0=st3[:, :nb, :], in1=ew_b, op=ALU.mult)
                if parts < 3:
                    continue
                for w in range(g0, g1):
                    psa = ps_agg.tile([128, D], f32, tag="psa")
                    for b in win_blocks[w]:
                        j = int(blk_sw[b]) % 4
                        # skip_group_check: sim's psum zero-region bookkeeping
                        # mis-addresses partition-sliced outputs
                        nc.tensor.matmul(
                            out=psa[j * SW : (j + 1) * SW, :],
                            lhsT=st3[:, b - blo, :],
                            rhs=msg3[:, b - blo, 0:D],
                            start=bool(block_first[b]),
                            stop=bool(block_last[b]),
                            tile_position=(0, j * SW),
                            skip_group_check=True,
                        )
                    epilogue(w, psa)

        # ---- layer 1 epilogue: z2s_w = dinv * (relu(dinv*agg @ W1) @ W2)
        def epi1(w, psa):
            agg = ep_p.tile([128, D], f32, tag="agg")
            nc.vector.tensor_scalar(
                out=agg[:], in0=psa[:], scalar1=dinv_l[:, w : w + 1], scalar2=None,
                op0=ALU.mult)
            pst = ps_eps.tile([64, 128], f32, tag="pst")
            nc.tensor.transpose(out=pst[:], in_=agg[:], identity=sb_ident[:])
            aggT = ep_p.tile([64, 128], f32, tag="aggT")
            nc.scalar.activation(out=aggT[:], in_=pst[:], func=AF.Copy)
            psh = ps_eps.tile([64, 128], f32, tag="psh")
            nc.tensor.matmul(out=psh[:], lhsT=sb_w1[:], rhs=aggT[:], start=True, stop=True)
            h1T = ep_p.tile([64, 128], f16, tag="h1T")
            nc.scalar.activation(out=h1T[:], in_=psh[:], func=AF.Relu)
            psz = ps_eps.tile([128, D], f32, tag="psz")
            nc.tensor.matmul(out=psz[:], lhsT=h1T[:], rhs=sb_w2h[:], start=True, stop=True)
            z2 = ep_p.tile([128, D], f16, tag="z2")
            nc.vector.tensor_scalar(
                out=z2[:], in0=psz[:], scalar1=dinv_l[:, w : w + 1], scalar2=None,
                op0=ALU.mult)
            nc.sync.dma_start(out=t_z2s[w * WIN : (w + 1) * WIN, 0:D], in_=z2[:])

        stage = int(os.environ.get("GCN_STAGE", "3"))
        if stage >= 1:
            layer(t_xp, epi1)
        if stage >= 2:
            nc.gpsimd.collective_compute(
                "AllGather",
                mybir.AluOpType.bypass,
                ins=[t_z2s[:, :]],
                outs=[t_z2t[:, :]],
                replica_groups=[list(range(G))],
            )

        # ---- layer 2 epilogue: h2 = relu(dinv*agg2); readout MLP
        def epi2(w, psa):
            h2r = ep_p.tile([128, H], f16, tag="h2r")
            nc.vector.tensor_scalar(
                out=h2r[:], in0=psa[:], scalar1=dinv_l[:, w : w + 1], scalar2=0.0,
                op0=ALU.mult, op1=ALU.max)
            nc.sync.dma_start(out=t_h2o[w * WIN : (w + 1) * WIN, :], in_=h2r[:])
            pst = ps_eps.tile([64, 128], f16, tag="pst")
            nc.tensor.transpose(out=pst[:], in_=h2r[:], identity=sb_ident16[:])
            h2T = ep_p.tile([64, 128], f16, tag="h1T")
            nc.scalar.activation(out=h2T[:], in_=pst[:], func=AF.Copy)
            pst1 = ps_eps.tile([64, 128], f32, tag="pst1")
            nc.tensor.matmul(out=pst1[:], lhsT=sb_a1h[:], rhs=h2T[:], start=True, stop=True)
            t1T = ep_p.tile([64, 128], f16, tag="t1T")
            nc.scalar.activation(
                out=t1T[:], in_=pst1[:], func=AF.Relu, bias=sb_b1f[:, 0:1])
            psr = ps_eps.tile([128, OUTF], f32, tag="psr")
            nc.tensor.matmul(out=psr[:], lhsT=t1T[:], rhs=sb_a2h[:], start=True, stop=True)
            rs = ep_p.tile([128, OUTF], f16, tag="rs")
            nc.vector.tensor_tensor(out=rs[:], in0=psr[:], in1=sb_b2r[:], op=ALU.add)
            nc.sync.dma_start(out=t_ro[w * WIN : (w + 1) * WIN, :], in_=rs[:])

        if stage >= 3:
            layer(t_z2t, epi2)

    nc.compile()
    return nc


def make_in_maps(cfg, core_arrays, x16, csr_pt, weights):
    W1, W2, A1, b1, A2, b2 = [np.asarray(w, np.float32) for w in weights]
    shared = dict(
        x16=x16,
        csrf=csr_pt,
        iota=np.tile(np.arange(SW, dtype=np.float16), (128, 1)),
        ident=np.eye(128, dtype=np.float32),
        ident16=np.eye(128, dtype=np.float16),
        w1f=W1,
        w2h=W2.astype(np.float16),
        a1h=A1.astype(np.float16),
        b1f=b1.reshape(H, 1),
        a2h=A2.astype(np.float16),
        b2r=np.tile(b2.reshape(1, OUTF), (128, 1)),
    )
    in_maps = []
    for k in range(G):
        m = dict(shared)
        ca = core_arrays[k]
        m["csrl"] = ca["csrl"]
        m["aidx"] = ca["aidx"]
        m["acol"] = ca["acol"]
        m["aew"] = ca["aew"]
        in_maps.append(m)
    return in_maps


_LAST_RESULTS = {}
_PROG_CACHE = {}


def kernel(x, edge_index, edge_weight, W1, W2, A1, b1, A2, b2):
    x = np.asarray(x, np.float32)
    N = x.shape[0]
    cfg, core_arrays, x16, csr_pt = host_prep(x, edge_index, edge_weight)
    key = (N, cfg["TOTBLK"], cfg["K1"], bytes(np.asarray(cfg["blk_sw"]).data))
    nc = _PROG_CACHE.get(key)
    if nc is None:
        nc = build_program(cfg)
        _PROG_CACHE.clear()
        _PROG_CACHE[key] = nc
    in_maps = make_in_maps(cfg, core_arrays, x16, csr_pt, (W1, W2, A1, b1, A2, b2))

    from concourse import bass_utils

    trace = bool(os.environ.get("GCN_TRACE"))
    res = bass_utils.run_bass_kernel_spmd(
        nc, in_maps, core_ids=list(range(G)), trace=trace
    )
    _LAST_RESULTS["exec_time_ns"] = res.exec_time_ns
    _LAST_RESULTS["results"] = res

    NLOC = cfg["NLOC"]
    h_parts, r_parts = [], []
    for k in range(G):
        n_real = min(NLOC, max(0, N - k * NLOC))
        h_parts.append(res.results[k]["h2o"][:n_real])
        r_parts.append(res.results[k]["ro"][:n_real])
    h = np.concatenate(h_parts, axis=0).astype(np.float32)
    r = np.concatenate(r_parts, axis=0).reshape(N, HOR, OUT).astype(np.float32)
    return (r, h)


# revision 29
# speedup vs baseline: 1.5979x; 1.1641x over previous
"""GCN decoder (2x GCNConv + MLP readout) on 8 Trainium2 NeuronCores.

Sharding: nodes are partitioned across the 8 cores by target-node range
(graph parallel). Edges are routed (on host) to the core that owns their
target node and sorted by target. Each core aggregates messages for its
own nodes with dma_gather + selection-matrix matmuls (scatter-add becomes
dense PE work); the layer-2 feature table is exchanged with one AllGather.

dma_gather uses int16 indices over 256B elements, so the feature tables
are stored as fp16 rows padded to 128 columns (256B) and edges are grouped
into blocks whose sources all come from one quarter of the table
(quarter size 25088 < int16 range).

self-contained: hardcodes the problem geometry, imports only installed
packages (numpy / concourse).
"""

import os
from contextlib import ExitStack

import numpy as np

# ---------------- fixed geometry ----------------
G = 8          # cores
NQ = 4         # table quarters (int16 index range for dma_gather)
WIN = 128      # nodes per PSUM accumulation window
SW = 32        # nodes per matmul output slice (sub-window)
KBLK = 128     # edge slots per matmul block
D = 64         # input feature dim
DP = 128       # padded row width (256B rows for dma_gather)
H = 64         # hidden dim
OUT = 32
HOR = 12
OUTF = OUT * HOR

# batching knobs
NB_DEG = 8     # windows per deg-reduce batch
NB_XP = 8      # windows per x-prescale batch
WG = 4         # windows per gather batch


def _cdiv(a, b):
    return (a + b - 1) // b


def _group_positions(sorted_group_ids, n_groups):
    counts = np.bincount(sorted_group_ids, minlength=n_groups)
    offsets = np.concatenate([[0], np.cumsum(counts)[:-1]])
    return np.arange(len(sorted_group_ids)) - offsets[sorted_group_ids], counts, offsets


def host_prep(x, edge_index, edge_weight):
    """Route/sort/pad edges, build CSR + slot arrays + static block schedule."""
    x = np.asarray(x, np.float32)
    edge_index = np.asarray(edge_index).astype(np.int64)
    edge_weight = np.asarray(edge_weight, np.float32)

    N = x.shape[0]
    NLOC = _cdiv(N, G)
    NPAD = _cdiv(NLOC, WIN) * WIN
    NTOT = G * NPAD
    QROWS = NTOT // NQ
    NWINL = NPAD // WIN
    NWINF = NTOT // WIN
    NSW = NPAD // SW
    assert NLOC < NPAD, "need dummy rows for gather padding"
    assert QROWS < 32768, "quarter must fit int16 index range"

    loop = np.arange(N, dtype=np.int64)
    row = np.concatenate([edge_index[0], loop])
    col = np.concatenate([edge_index[1], loop])
    ew = np.concatenate([edge_weight, np.ones(N, np.float32)])

    core_of = np.minimum(col // NLOC, G - 1)
    rcore = np.minimum(row // NLOC, G - 1)
    gid_row = (rcore * NPAD + (row - rcore * NLOC)).astype(np.int64)

    # ---- full-graph CSR of edge weights, gid order, for on-device degrees
    gcol = (core_of * NPAD + (col - core_of * NLOC)).astype(np.int64)
    order = np.argsort(gcol, kind="stable")
    gcol_s = gcol[order]
    pos, counts_all, _ = _group_positions(gcol_s, NTOT)
    K1 = max(8, int(counts_all.max()))
    K1 = _cdiv(K1, 8) * 8
    csr = np.zeros((NTOT, K1), np.float16)
    csr[gcol_s, pos] = ew[order].astype(np.float16)
    dummy_mask = np.zeros(NTOT, bool)
    for k in range(G):
        n_real = min(NLOC, max(0, N - k * NLOC))
        dummy_mask[k * NPAD + n_real : (k + 1) * NPAD] = True
    csr[dummy_mask, 0] = 1.0
    csr_pt = np.ascontiguousarray(csr.reshape(NWINF, WIN, K1).transpose(0, 2, 1))

    # ---- per-core edges sorted by (target sub-window, source quarter)
    per_core = []
    counts_swq = np.zeros((G, NSW * NQ), np.int64)
    for k in range(G):
        m = core_of == k
        e_gr = gid_row[m]
        e_cl = (col[m] - k * NLOC).astype(np.int64)
        e_w = ew[m].astype(np.float16)
        e_q = e_gr // QROWS
        key = (e_cl // SW) * NQ + e_q
        o = np.argsort(key, kind="stable")
        e_gr, e_cl, e_w, key = e_gr[o], e_cl[o], e_w[o], key[o]
        counts_swq[k] = np.bincount(key, minlength=NSW * NQ)
        per_core.append((e_gr, e_cl, e_w, key))

    # blocks per (sw, q): shared static schedule; q of each block is static
    Bq = _cdiv(counts_swq.max(axis=0), KBLK).reshape(NSW, NQ)  # may be 0
    # every sub-window needs >=1 block so its psum slice gets initialized;
    # self-loops guarantee the own-core quarter is nonempty, but be safe:
    for swi in range(NSW):
        if Bq[swi].sum() == 0:
            Bq[swi, 0] = 1

    # global block order: per gather batch (WG windows), quarter-major
    batches = []
    blk_sw = []
    blk_q = []
    blk_of_swq = {}
    for g0 in range(0, NWINL, WG):
        g1 = min(g0 + WG, NWINL)
        b_start = len(blk_sw)
        q_ranges = []
        for q in range(NQ):
            q_lo = len(blk_sw)
            for w in range(g0, g1):
                for swi in range(w * 4, w * 4 + 4):
                    nbq = int(Bq[swi, q])
                    if nbq:
                        blk_of_swq[(swi, q)] = (len(blk_sw), nbq)
                        for _ in range(nbq):
                            blk_sw.append(swi)
                            blk_q.append(q)
            q_ranges.append((q_lo, len(blk_sw)))
        batches.append(dict(g0=g0, g1=g1, blo=b_start, bhi=len(blk_sw),
                            q_ranges=q_ranges))
    TOTBLK = len(blk_sw)
    SLOTS = TOTBLK * KBLK
    blk_sw = np.asarray(blk_sw)
    blk_q = np.asarray(blk_q)
    # first/last block of each sub-window in global order
    block_first = np.zeros(TOTBLK, bool)
    block_last = np.zeros(TOTBLK, bool)
    seen = set()
    last_of = {}
    for b in range(TOTBLK):
        swi = int(blk_sw[b])
        if swi not in seen:
            seen.add(swi)
            block_first[b] = True
        last_of[swi] = b
    for b in last_of.values():
        block_last[b] = True
    win_blocks = [[] for _ in range(NWINL)]
    for b in range(TOTBLK):
        win_blocks[int(blk_sw[b]) // 4].append(b)

    core_arrays = []
    for k in range(G):
        e_gr, e_cl, e_w, key = per_core[k]
        # slot position for each edge: its (sw, q) block region + within-pos
        wpos, _, _ = _group_positions(key, NSW * NQ)
        base = np.zeros(NSW * NQ, np.int64)
        for (swi, q), (b0, nbq) in blk_of_swq.items():
            base[swi * NQ + q] = b0 * KBLK
        slot = base[key] + wpos
        # padding -> quarter-local dummy row (all-zero, core 2q's last row)
        idxs = np.empty(SLOTS, np.int16)
        q_of_slot = blk_q[np.arange(SLOTS) // KBLK]
        idxs[:] = (NPAD - 1)  # dummy local idx within any quarter
        idxs_local_dummy = np.full(SLOTS, NPAD - 1, np.int64)
        idxs_local = idxs_local_dummy.copy()
        idxs_local[slot] = e_gr - (e_gr // QROWS) * QROWS
        assert idxs_local.max() < 32768
        col_slots = np.zeros(SLOTS, np.float16)
        ew_slots = np.zeros(SLOTS, np.float16)
        col_slots[slot] = (e_cl % SW).astype(np.float16)
        ew_slots[slot] = e_w
        # dma_gather idx layout: stream i reads idx[i%16, i//16], 16-partition
        # wrapped, replicated across the 8 q7 core groups
        a_idx16 = np.ascontiguousarray(
            idxs_local.astype(np.int16).reshape(-1, 16).T)  # [16, SLOTS/16]
        a_col = np.ascontiguousarray(col_slots.reshape(TOTBLK, KBLK).T)
        a_ew = np.ascontiguousarray(ew_slots.reshape(TOTBLK, KBLK).T)
        csr_l = np.ascontiguousarray(
            csr[k * NPAD : (k + 1) * NPAD].reshape(NWINL, WIN, K1).transpose(0, 2, 1)
        )
        core_arrays.append(dict(aidx=a_idx16, acol=a_col, aew=a_ew, csrl=csr_l))

    x16 = np.zeros((NTOT, D), np.float16)
    allg = (np.minimum(loop // NLOC, G - 1) * NPAD
            + (loop - np.minimum(loop // NLOC, G - 1) * NLOC)).astype(np.int64)
    x16[allg] = x.astype(np.float16)

    cfg = dict(
        N=N, NLOC=NLOC, NPAD=NPAD, NTOT=NTOT, QROWS=QROWS, NWINL=NWINL,
        NWINF=NWINF, NSW=NSW, K1=K1, TOTBLK=TOTBLK,
        blk_sw=blk_sw, block_first=block_first, block_last=block_last,
        batches=batches, win_blocks=win_blocks,
    )
    return cfg, core_arrays, x16, csr_pt


def build_program(cfg):
    import concourse.bacc as bacc
    import concourse.mybir as mybir
    import concourse.tile as tile

    f32 = mybir.dt.float32
    f16 = mybir.dt.float16
    i16 = mybir.dt.int16
    ALU = mybir.AluOpType
    AF = mybir.ActivationFunctionType

    NTOT, NPAD, QROWS = cfg["NTOT"], cfg["NPAD"], cfg["QROWS"]
    NWINL, NWINF, K1 = cfg["NWINL"], cfg["NWINF"], cfg["K1"]
    TOTBLK = cfg["TOTBLK"]
    blk_sw = cfg["blk_sw"]
    block_first = cfg["block_first"]
    block_last = cfg["block_last"]
    batches = cfg["batches"]
    win_blocks = cfg["win_blocks"]

    nc = bacc.Bacc("TRN2", num_devices=G)

    t_x16 = nc.dram_tensor("x16", [NTOT, D], f16, kind="ExternalInput")
    t_csrl = nc.dram_tensor("csrl", [NWINL, K1, WIN], f16, kind="ExternalInput")
    t_aidx = nc.dram_tensor("aidx", [16, TOTBLK * 8], i16, kind="ExternalInput")
    t_acol = nc.dram_tensor("acol", [KBLK, TOTBLK], f16, kind="ExternalInput")
    t_aew = nc.dram_tensor("aew", [KBLK, TOTBLK], f16, kind="ExternalInput")
    t_iota = nc.dram_tensor("iota", [128, SW], f16, kind="ExternalInput")
    t_ident = nc.dram_tensor("ident", [128, 128], f32, kind="ExternalInput")
    t_ident16 = nc.dram_tensor("ident16", [128, 128], f16, kind="ExternalInput")
    t_w1 = nc.dram_tensor("w1f", [D, H], f32, kind="ExternalInput")
    t_w2h = nc.dram_tensor("w2h", [H, H], f16, kind="ExternalInput")
    t_a1h = nc.dram_tensor("a1h", [H, H], f16, kind="ExternalInput")
    t_b1f = nc.dram_tensor("b1f", [H, 1], f32, kind="ExternalInput")
    t_a2h = nc.dram_tensor("a2h", [H, OUTF], f16, kind="ExternalInput")
    t_b2r = nc.dram_tensor("b2r", [128, OUTF], f32, kind="ExternalInput")

    # gather tables: fp16 rows padded to DP columns (256B elements).
    # right halves are never written nor read.
    t_xp = nc.dram_tensor("xp", [NTOT, DP], f16)
    t_z2t = nc.dram_tensor("z2t", [NTOT, DP], f16, addr_space="Shared")
    t_z2s = nc.dram_tensor("z2s", [NPAD, DP], f16)
    t_dvs = nc.dram_tensor("dvs", [NPAD, 1], f32)
    t_dvt = nc.dram_tensor("dvt", [NTOT, 1], f32, addr_space="Shared")

    t_h2o = nc.dram_tensor("h2o", [NPAD, H], f16, kind="ExternalOutput")
    t_ro = nc.dram_tensor("ro", [NPAD, OUTF], f16, kind="ExternalOutput")

    with tile.TileContext(nc) as tc, ExitStack() as ctx:
        const = ctx.enter_context(tc.tile_pool(name="const", bufs=1))
        persist = ctx.enter_context(tc.tile_pool(name="persist", bufs=1))
        edgep = ctx.enter_context(tc.tile_pool(name="edgep", bufs=1))
        csr_p = ctx.enter_context(tc.tile_pool(name="csrp", bufs=2))
        xp_p = ctx.enter_context(tc.tile_pool(name="xpp", bufs=3))
        idx_p = ctx.enter_context(tc.tile_pool(name="idxp", bufs=3))
        msg_p = ctx.enter_context(tc.tile_pool(name="msgp", bufs=2))
        s_p = ctx.enter_context(tc.tile_pool(name="sp", bufs=2))
        ep_p = ctx.enter_context(tc.tile_pool(name="epp", bufs=2))
        ps_agg = ctx.enter_context(tc.tile_pool(name="psagg", bufs=2, space="PSUM"))
        ps_eps = ctx.enter_context(tc.tile_pool(name="pseps", bufs=1, space="PSUM"))

        def _load(pool, t, shape, dtype):
            tl = pool.tile(shape, dtype, tag=t.name + "_sb")
            nc.sync.dma_start(out=tl[:], in_=t[:, :])
            return tl

        sb_iota = _load(const, t_iota, [128, SW], f16)
        sb_ident = _load(const, t_ident, [128, 128], f32)
        sb_ident16 = _load(const, t_ident16, [128, 128], f16)
        sb_w1 = _load(const, t_w1, [D, H], f32)
        sb_w2h = _load(const, t_w2h, [H, H], f16)
        sb_a1h = _load(const, t_a1h, [H, H], f16)
        sb_b1f = _load(const, t_b1f, [H, 1], f32)
        sb_a2h = _load(const, t_a2h, [H, OUTF], f16)
        sb_b2r = _load(const, t_b2r, [128, OUTF], f32)
        sb_acol = edgep.tile([KBLK, TOTBLK], f16, tag="acol_sb")
        nc.sync.dma_start(out=sb_acol[:], in_=t_acol[:, :])
        sb_aew = edgep.tile([KBLK, TOTBLK], f16, tag="aew_sb")
        nc.sync.dma_start(out=sb_aew[:], in_=t_aew[:, :])

        # ---- degrees -> dinv
        deg_l = persist.tile([128, NWINL], f32, tag="degl")
        dinv_f16 = persist.tile([128, NWINF], f16, tag="dinvf16")
        dinv_l = persist.tile([128, NWINL], f32, tag="dinvl")

        def deg_phase(t_csr, nwin, out_deg):
            for b0 in range(0, nwin, NB_DEG):
                nb = min(NB_DEG, nwin - b0)
                tl = csr_p.tile([128, NB_DEG * K1], f16, tag="csr_t")
                src = t_csr[b0 : b0 + nb, :, :].rearrange("b k p -> p b k")
                dst3 = tl[:].rearrange("p (b k) -> p b k", k=K1)
                nc.sync.dma_start(out=dst3[:, :nb, :], in_=src)
                nc.vector.tensor_reduce(
                    out=out_deg[:, b0 : b0 + nb], in_=dst3[:, :nb, :],
                    axis=mybir.AxisListType.X, op=ALU.add,
                )

        deg_phase(t_csrl, NWINL, deg_l)
        rec_l = persist.tile([128, NWINL], f32, tag="recl")
        nc.vector.reciprocal(out=rec_l[:], in_=deg_l[:])
        nc.scalar.sqrt(out=dinv_l[:], in_=rec_l[:])
        # exchange per-shard dinv -> full table (tiny collective)
        nc.sync.dma_start(
            out=t_dvs[:, :].rearrange("(w p) o -> p w o", p=WIN),
            in_=dinv_l[:].unsqueeze(2))
        nc.gpsimd.collective_compute(
            "AllGather", mybir.AluOpType.bypass,
            ins=[t_dvs[:, :]], outs=[t_dvt[:, :]],
            replica_groups=[list(range(G))])
        dinv_f = persist.tile([128, NWINF], f32, tag="dinvf")
        nc.sync.dma_start(
            out=dinv_f[:].unsqueeze(2),
            in_=t_dvt[:, :].rearrange("(w p) o -> p w o", p=WIN))
        nc.vector.tensor_copy(out=dinv_f16[:], in_=dinv_f[:])

        # ---- xp = dinv * x16 (full padded table, left halves only)
        for b0 in range(0, NWINF, NB_XP):
            nb = min(NB_XP, NWINF - b0)
            xt = xp_p.tile([128, NB_XP * D], f16, tag="xt")
            src = t_x16[b0 * WIN : (b0 + nb) * WIN, :].rearrange(
                "(b p) d -> p b d", p=WIN)
            xt3 = xt[:].rearrange("p (b d) -> p b d", d=D)
            nc.sync.dma_start(out=xt3[:, :nb, :], in_=src)
            xs = xp_p.tile([128, NB_XP * D], f16, tag="xs")
            xs3 = xs[:].rearrange("p (b d) -> p b d", d=D)
            dv = dinv_f16[:, b0 : b0 + nb].unsqueeze(2).to_broadcast([128, nb, D])
            nc.vector.tensor_tensor(
                out=xs3[:, :nb, :], in0=xt3[:, :nb, :], in1=dv, op=ALU.mult)
            dst = t_xp[b0 * WIN : (b0 + nb) * WIN, 0:D].rearrange(
                "(b p) d -> p b d", p=WIN)
            nc.sync.dma_start(out=dst, in_=xs3[:, :nb, :])

        max_batch_blk = max(bt["bhi"] - bt["blo"] for bt in batches)
        max_win_blk = max(len(bl) for bl in win_blocks)

        # ---- one aggregation layer over a padded table
        def layer(table_t, epilogue):
            for bt in batches:
                g0, g1 = bt["g0"], bt["g1"]
                blo, bhi = bt["blo"], bt["bhi"]
                nb = bhi - blo
                idx_sb = idx_p.tile([128, max_batch_blk * 8], i16, tag="idxt")
                for gk in range(8):
                    nc.sync.dma_start(
                        out=idx_sb[gk * 16 : (gk + 1) * 16, : nb * 8],
                        in_=t_aidx[:, blo * 8 : bhi * 8])
                msg = msg_p.tile([128, max_batch_blk * DP], f16, tag="msg")
                msg3 = msg[:].rearrange("p (b d) -> p b d", d=DP)
                for q, (qlo, qhi) in enumerate(bt["q_ranges"]):
                    # dma_gather misbehaves above ~1024 idxs/call; chunk it
                    for c0 in range(qlo, qhi, 8):
                        c1 = min(c0 + 8, qhi)
                        ncb = c1 - c0
                        nc.gpsimd.dma_gather(
                            out_ap=msg3[:, c0 - blo : c1 - blo, :],
                            in_ap=table_t[q * QROWS : (q + 1) * QROWS, :],
                            idxs_ap=idx_sb[:, (c0 - blo) * 8 : (c1 - blo) * 8],
                            num_idxs=ncb * KBLK,
                            num_idxs_reg=ncb * KBLK,
                            elem_size=DP,
                        )
                st = s_p.tile([128, max_batch_blk * SW], f16, tag="stile")
                st3 = st[:].rearrange("p (b t) -> p b t", t=SW)
                io_b = sb_iota[:, :].unsqueeze(1).to_broadcast([128, nb, SW])
                cl_b = sb_acol[:, blo:bhi].unsqueeze(2).to_broadcast([128, nb, SW])
                ew_b = sb_aew[:, blo:bhi].unsqueeze(2).to_broadcast([128, nb, SW])
                parts = int(os.environ.get("GCN_L1PARTS", "3"))
                if parts < 2:
                    continue
                nc.vector.tensor_tensor(
                    out=st3[:, :nb, :], in0=io_b, in1=cl_b, op=ALU.is_equal)
                nc.vector.tensor_tensor(
                    out=st3[:, :nb, :], in0=st3[:, :nb, :], in1=ew_b, op=ALU.mult)
                if parts < 3:
                    continue
                for w in range(g0, g1):
                    psa = ps_agg.tile([128, D], f32, tag="psa")
                    for b in win_blocks[w]:
                        j = int(blk_sw[b]) % 4
                        # skip_group_check: sim's psum zero-region bookkeeping
                        # mis-addresses partition-sliced outputs
                        nc.tensor.matmul(
                            out=psa[j * SW : (j + 1) * SW, :],
                            lhsT=st3[:, b - blo, :],
                            rhs=msg3[:, b - blo, 0:D],
                            start=bool(block_first[b]),
                            stop=bool(block_last[b]),
                            tile_position=(0, j * SW),
                            skip_group_check=True,
                        )
                    epilogue(w, psa)

        # ---- layer 1 epilogue: z2s_w = dinv * (relu(dinv*agg @ W1) @ W2)
        def epi1(w, psa):
            agg = ep_p.tile([128, D], f32, tag="agg")
            nc.vector.tensor_scalar(
                out=agg[:], in0=psa[:], scalar1=dinv_l[:, w : w + 1], scalar2=None,
                op0=ALU.mult)
            pst = ps_eps.tile([64, 128], f32, tag="pst")
            nc.tensor.transpose(out=pst[:], in_=agg[:], identity=sb_ident[:])
            aggT = ep_p.tile([64, 128], f32, tag="aggT")
            nc.scalar.activation(out=aggT[:], in_=pst[:], func=AF.Copy)
            psh = ps_eps.tile([64, 128], f32, tag="psh")
            nc.tensor.matmul(out=psh[:], lhsT=sb_w1[:], rhs=aggT[:], start=True, stop=True)
            h1T = ep_p.tile([64, 128], f16, tag="h1T")
            nc.scalar.activation(out=h1T[:], in_=psh[:], func=AF.Relu)
            psz = ps_eps.tile([128, D], f32, tag="psz")
            nc.tensor.matmul(out=psz[:], lhsT=h1T[:], rhs=sb_w2h[:], start=True, stop=True)
            z2 = ep_p.tile([128, D], f16, tag="z2")
            nc.vector.tensor_scalar(
                out=z2[:], in0=psz[:], scalar1=dinv_l[:, w : w + 1], scalar2=None,
                op0=ALU.mult)
            nc.sync.dma_start(out=t_z2s[w * WIN : (w + 1) * WIN, 0:D], in_=z2[:])

        stage = int(os.environ.get("GCN_STAGE", "3"))
        if stage >= 1:
            layer(t_xp, epi1)
        if stage >= 2:
            nc.gpsimd.collective_compute(
                "AllGather",
                mybir.AluOpType.bypass,
                ins=[t_z2s[:, :]],
                outs=[t_z2t[:, :]],
                replica_groups=[list(range(G))],
            )

        # ---- layer 2 epilogue: h2 = relu(dinv*agg2); readout MLP
        def epi2(w, psa):
            h2r = ep_p.tile([128, H], f16, tag="h2r")
            nc.vector.tensor_scalar(
                out=h2r[:], in0=psa[:], scalar1=dinv_l[:, w : w + 1], scalar2=0.0,
                op0=ALU.mult, op1=ALU.max)
            nc.sync.dma_start(out=t_h2o[w * WIN : (w + 1) * WIN, :], in_=h2r[:])
            pst = ps_eps.tile([64, 128], f16, tag="pst")
            nc.tensor.transpose(out=pst[:], in_=h2r[:], identity=sb_ident16[:])
            h2T = ep_p.tile([64, 128], f16, tag="h1T")
            nc.scalar.activation(out=h2T[:], in_=pst[:], func=AF.Copy)
            pst1 = ps_eps.tile([64, 128], f32, tag="pst1")
            nc.tensor.matmul(out=pst1[:], lhsT=sb_a1h[:], rhs=h2T[:], start=True, stop=True)
            t1T = ep_p.tile([64, 128], f16, tag="t1T")
            nc.scalar.activation(
                out=t1T[:], in_=pst1[:], func=AF.Relu, bias=sb_b1f[:, 0:1])
            psr = ps_eps.tile([128, OUTF], f32, tag="psr")
            nc.tensor.matmul(out=psr[:], lhsT=t1T[:], rhs=sb_a2h[:], start=True, stop=True)
            rs = ep_p.tile([128, OUTF], f16, tag="rs")
            nc.vector.tensor_tensor(out=rs[:], in0=psr[:], in1=sb_b2r[:], op=ALU.add)
            nc.sync.dma_start(out=t_ro[w * WIN : (w + 1) * WIN, :], in_=rs[:])

        if stage >= 3:
            layer(t_z2t, epi2)

    nc.compile()
    return nc


def make_in_maps(cfg, core_arrays, x16, csr_pt, weights):
    W1, W2, A1, b1, A2, b2 = [np.asarray(w, np.float32) for w in weights]
    shared = dict(
        x16=x16,
        iota=np.tile(np.arange(SW, dtype=np.float16), (128, 1)),
        ident=np.eye(128, dtype=np.float32),
        ident16=np.eye(128, dtype=np.float16),
        w1f=W1,
        w2h=W2.astype(np.float16),
        a1h=A1.astype(np.float16),
        b1f=b1.reshape(H, 1),
        a2h=A2.astype(np.float16),
        b2r=np.tile(b2.reshape(1, OUTF), (128, 1)),
    )
    in_maps = []
    for k in range(G):
        m = dict(shared)
        ca = core_arrays[k]
        m["csrl"] = ca["csrl"]
        m["aidx"] = ca["aidx"]
        m["acol"] = ca["acol"]
        m["aew"] = ca["aew"]
        in_maps.append(m)
    return in_maps


_LAST_RESULTS = {}
_PROG_CACHE = {}


def kernel(x, edge_index, edge_weight, W1, W2, A1, b1, A2, b2):
    x = np.asarray(x, np.float32)
    N = x.shape[0]
    cfg, core_arrays, x16, csr_pt = host_prep(x, edge_index, edge_weight)
    key = (N, cfg["TOTBLK"], cfg["K1"], bytes(np.asarray(cfg["blk_sw"]).data))
    nc = _PROG_CACHE.get(key)
    if nc is None:
        nc = build_program(cfg)
        _PROG_CACHE.clear()
        _PROG_CACHE[key] = nc
    in_maps = make_in_maps(cfg, core_arrays, x16, csr_pt, (W1, W2, A1, b1, A2, b2))

    from concourse import bass_utils

    trace = bool(os.environ.get("GCN_TRACE"))
    res = bass_utils.run_bass_kernel_spmd(
        nc, in_maps, core_ids=list(range(G)), trace=trace
    )
    _LAST_RESULTS["exec_time_ns"] = res.exec_time_ns
    _LAST_RESULTS["results"] = res

    NLOC = cfg["NLOC"]
    h_parts, r_parts = [], []
    for k in range(G):
        n_real = min(NLOC, max(0, N - k * NLOC))
        h_parts.append(res.results[k]["h2o"][:n_real])
        r_parts.append(res.results[k]["ro"][:n_real])
    h = np.concatenate(h_parts, axis=0).astype(np.float32)
    r = np.concatenate(r_parts, axis=0).reshape(N, HOR, OUT).astype(np.float32)
    return (r, h)
